# revision 1
# baseline (speedup 1.0000x reference)
"""Depth-aware 3x3 conv (Wang & Neumann depth-similarity modulated conv) on
8 Trainium2 NeuronCores, batch-parallel (1 image per core).

out[b,o,h,w] = sum_{c,k} weight[o,c,k] * fd[b,k,h,w] * xpatch[b,c,k,h,w] + bias
fd[k,p] = exp(-8.3 * |depth[p + delta_k] - depth[p]|)   (zero-padded patches)

Device-side formulation per core (image [64, 256, 256] fp16):
- Padded plane flattened: q = (h+1)*258 + (w+1), NP = 258*258.
- 9 taps delta(kh,kw) = (kh-1)*258 + (kw-1), paired so each pair's two taps
  live in partition halves of one [128, N] "modulated moving" tile:
    pair tiles: T1 = [x ; x@+1], T2 = [x ; x@+256]  (upper half pre-shifted)
    pairs: A=(t0,t1) on T1, B=(t7,t8) on T1, C=(t2,t3) on T2, D=(t5,t6) on T2
    center t4 unmodulated (fd=1) from T1 lower rows.
- fd rows computed packed [72 = 9 taps x 8 segs, 512] (sub/abs on DVE, exp on
  ACT -> fp16), written to a DRAM scratch grid, then replicated across 64
  partitions via partition-step-0 DMA reads (DMA xbar fanout; cheap).
- DVE tensor_tensor builds M''_g = x2_g o fd_rep_g at [128, 2308] (fp16 2x).
- PE: per 512-px group, 5 matmuls (4 pairs K=128 + center K=64) accumulate
  into one PSUM bank; ScalarE evicts with per-partition bias -> fp16.
"""
import numpy as np

import concourse.bacc as bacc
import concourse.bass as bass
import concourse.mybir as mybir
import concourse.tile as tile
from concourse.bass_utils import run_bass_kernel_spmd

F16 = mybir.dt.float16
F32 = mybir.dt.float32

B, C, H, W = 8, 64, 256, 256
Hp, Wp = H + 2, W + 2          # 258
NP = Hp * Wp                   # 66564
ALPHA = 8.3

CH = 4096                      # output pixels per chunk
NCHUNK = -(-NP // CH)          # 17 (out grid 69632, host slices)
WT = CH + 520                  # x2 tile width (halo for tap shifts)
HALF = WT // 2                 # 2308 (even)
SEG, SEGW = 8, CH // 8         # fd packing: [72, 512]

# line slacks (elements)
XSL, XSH = 512, 4608           # x line: reads [q0-260, q0+CH+516)
DSL, DSH = 512, 4608           # depth line
XW = XSL + NP + XSH
DW = DSL + NP + DSH
OUTW = NCHUNK * CH             # 69632

FDS = 1024                     # fd region slack lo
W1 = FDS + WT + 1024           # fd region width (per parity), reads < FDS+4615
FDW = 2 * W1

# tap id t = kh*3+kw, delta = (kh-1)*258 + (kw-1)
DELTA = [(kh - 1) * Wp + (kw - 1) for kh in range(3) for kw in range(3)]
# pairs (ta, tb): tb's shift baked into the tile's upper half
PAIRS = [(0, 1, 0), (7, 8, 0), (2, 3, 1), (5, 6, 1)]  # (ta, tb, tile_idx)
UPPER_SHIFT = [1, 256]  # T1, T2


def _build_nc():
    nc = bacc.Bacc("TRN2", target_bir_lowering=False, debug=False, num_devices=8)
    x_line = nc.declare_dram_parameter("x_line", [C, XW], F16, isOutput=False)
    d_line = nc.declare_dram_parameter("d_line", [1, DW], F32, isOutput=False)
    wts = nc.declare_dram_parameter("wts", [128, 5 * 64], F16, isOutput=False)
    bias = nc.declare_dram_parameter("bias", [64, 1], F32, isOutput=False)
    out_l = nc.declare_dram_parameter("out_line", [C, OUTW], F16, isOutput=True)

    x_t = x_line.ap().tensor
    d_t = d_line.ap().tensor
    fd_dram = nc.dram_tensor("fd_scratch", [9, FDW], F16)
    fd_t = fd_dram.ap().tensor

    with tile.TileContext(nc) as tc:
        with (
            tc.tile_pool(name="const", bufs=1) as cpool,
            tc.tile_pool(name="xt", bufs=4) as xpool,
            tc.tile_pool(name="fdgen", bufs=3) as gpool,
            tc.tile_pool(name="frep", bufs=10) as fpool,
            tc.tile_pool(name="mmod", bufs=6) as mpool,
            tc.tile_pool(name="ost", bufs=2) as opool,
            tc.tile_pool(name="ps", bufs=8, space="PSUM") as pspool,
        ):
            wt_sb = cpool.tile([128, 5 * 64], F16, tag="w")
            nc.sync.dma_start(wt_sb[:], wts[:])
            bias_sb = cpool.tile([64, 1], F32, tag="b")
            nc.sync.dma_start(bias_sb[:], bias[:])

            for i in range(NCHUNK):
                q0 = i * CH
                xbase = XSL + q0 - 260
                # ---- x2 tiles (lower = x@-260, upper = x@-260+shift) ----
                xts = []
                for ti in range(2):
                    xt = xpool.tile([128, WT], F16, tag="x")
                    nc.sync.dma_start(
                        xt[0:64, :],
                        bass.AP(x_t, xbase, [[XW, 64], [1, WT]]))
                    nc.sync.dma_start(
                        xt[64:128, :],
                        bass.AP(x_t, xbase + UPPER_SHIFT[ti],
                                [[XW, 64], [1, WT]]))
                    xts.append(xt)

                # ---- fd generation (packed [72, 512]) ----
                dp = gpool.tile([72, SEGW], F32, tag="dp")
                for kh in range(3):
                    nc.sync.dma_start(
                        dp[kh * 24:(kh + 1) * 24, :],
                        bass.AP(d_t, DSL + q0 - 259 + kh * Wp,
                                [[1, 3], [SEGW, SEG], [1, SEGW]]))
                dc = gpool.tile([72, SEGW], F32, tag="dc")
                nc.sync.dma_start(
                    dc[:],
                    bass.AP(d_t, DSL + q0,
                            [[0, 9], [SEGW, SEG], [1, SEGW]]))
                df = gpool.tile([72, SEGW], F32, tag="df")
                nc.vector.tensor_tensor(df[:], dp[:], dc[:],
                                        mybir.AluOpType.subtract)
                da = gpool.tile([72, SEGW], F32, tag="da")
                nc.scalar.activation(da[:], df[:],
                                     mybir.ActivationFunctionType.Abs)
                fdp = gpool.tile([72, SEGW], F16, tag="fdp")
                nc.scalar.activation(fdp[:], da[:],
                                     mybir.ActivationFunctionType.Exp,
                                     scale=-ALPHA)
                # ---- scatter to DRAM fd grid (per-parity region) ----
                reg = (i % 2) * W1
                for t9 in range(9):
                    nc.sync.dma_start(
                        bass.AP(fd_t, t9 * FDW + reg + FDS,
                                [[SEGW, SEG], [1, SEGW]]),
                        fdp[t9 * SEG:(t9 + 1) * SEG, :])

                # ---- per pair: replicate fd (step-0 DMA), modulate (DVE) ----
                mts = []
                for (ta, tb, ti) in PAIRS:
                    mt = mpool.tile([128, WT], F16, tag="m")
                    for h in range(2):
                        fr = fpool.tile([128, HALF], F16, tag="fr")
                        off = reg + FDS + h * HALF - 260 - DELTA[ta]
                        nc.sync.dma_start(
                            fr[0:64, :],
                            bass.AP(fd_t, ta * FDW + off,
                                    [[0, 64], [1, HALF]]))
                        nc.sync.dma_start(
                            fr[64:128, :],
                            bass.AP(fd_t, tb * FDW + off,
                                    [[0, 64], [1, HALF]]))
                        nc.vector.tensor_tensor(
                            mt[:, h * HALF:(h + 1) * HALF],
                            xts[ti][:, h * HALF:(h + 1) * HALF],
                            fr[:], mybir.AluOpType.mult)
                    mts.append(mt)

                # ---- matmuls + eviction ----
                ost = opool.tile([64, CH], F16, tag="o")
                for j in range(CH // 512):
                    ps = pspool.tile([64, 512], F32)
                    for g, (ta, tb, ti) in enumerate(PAIRS):
                        m0 = 260 + DELTA[ta]
                        nc.tensor.matmul(
                            ps[:], wt_sb[:, g * 64:(g + 1) * 64],
                            mts[g][:, m0 + j * 512: m0 + (j + 1) * 512],
                            start=(g == 0), stop=False)
                    nc.tensor.matmul(
                        ps[:], wt_sb[0:64, 256:320],
                        xts[0][0:64, 260 + j * 512: 260 + (j + 1) * 512],
                        start=False, stop=True)
                    nc.scalar.activation(
                        ost[:, j * 512:(j + 1) * 512], ps[:],
                        mybir.ActivationFunctionType.Identity,
                        bias=bias_sb[:], scale=1.0)
                nc.sync.dma_start(out_l[:, q0:q0 + CH], ost[:])
    nc.compile()
    return nc


_NC_CACHE = None


def _get_nc():
    global _NC_CACHE
    if _NC_CACHE is None:
        _NC_CACHE = _build_nc()
    return _NC_CACHE


def kernel(x, depth, weight, bias):
    x = np.asarray(x, dtype=np.float32)
    depth = np.asarray(depth, dtype=np.float32)
    weight = np.asarray(weight, dtype=np.float32)
    bias_np = np.asarray(bias, dtype=np.float32)

    # host prep
    xl = np.zeros((B, C, XW), np.float16)
    xpad = np.zeros((B, C, Hp, Wp), np.float32)
    xpad[:, :, 1:257, 1:257] = x
    xl[:, :, XSL:XSL + NP] = xpad.reshape(B, C, NP).astype(np.float16)

    dl = np.zeros((B, 1, DW), np.float32)
    dpad = np.zeros((B, Hp, Wp), np.float32)
    dpad[:, 1:257, 1:257] = depth[:, 0]
    dl[:, 0, DSL:DSL + NP] = dpad.reshape(B, NP)

    wts = np.zeros((128, 5 * 64), np.float16)
    for g, (ta, tb, _) in enumerate(PAIRS):
        # lhsT[c, o] = weight[o, c, kh, kw]
        wts[0:64, g * 64:(g + 1) * 64] = \
            weight[:, :, ta // 3, ta % 3].T.astype(np.float16)
        wts[64:128, g * 64:(g + 1) * 64] = \
            weight[:, :, tb // 3, tb % 3].T.astype(np.float16)
    wts[0:64, 256:320] = weight[:, :, 1, 1].T.astype(np.float16)

    bias_col = bias_np.reshape(64, 1)

    nc = _get_nc()
    in_maps = [
        {"x_line": xl[b], "d_line": dl[b], "wts": wts, "bias": bias_col}
        for b in range(B)
    ]
    res = run_bass_kernel_spmd(nc, in_maps, list(range(B)))

    out = np.empty((B, C, H, W), np.float32)
    for b in range(B):
        ol = res.results[b]["out_line"][:, :NP].astype(np.float32)
        out[b] = ol.reshape(C, Hp, Wp)[:, 1:257, 1:257]
    return out



# revision 2
# speedup vs baseline: 1.4638x; 1.4638x over previous
"""Depth-aware 3x3 conv (Wang & Neumann depth-similarity modulated conv) on
8 Trainium2 NeuronCores, batch-parallel (1 image per core).

out[b,o,h,w] = sum_{c,k} weight[o,c,k] * fd[b,k,h,w] * xpatch[b,c,k,h,w] + bias
fd[k,p] = exp(-8.3 * |depth[p + delta_k] - depth[p]|)   (zero-padded patches)

Device-side formulation per core (image [64, 256, 256] fp16):
- Padded plane flattened: q = (h+1)*258 + (w+1), NP = 258*258.
- 9 taps delta(kh,kw) = (kh-1)*258 + (kw-1), paired so each pair's two taps
  live in partition halves of one [128, N] tile:
    pair x tiles: T1 = [x ; x@+1], T2 = [x ; x@+256] (one 3-dim DMA each)
    pairs: A=(t0,t1) on T1, B=(t7,t8) on T1, C=(t2,t3) on T2, D=(t5,t6) on T2
    center t4 unmodulated (fd=1) from T1 lower rows.
- fd computed packed [72 = 9 taps x 8 segs, 512] (sub on DVE, abs+exp on
  ACT -> fp16), scattered to a DRAM grid in ONE DMA, then per pair ONE
  pixel-aligned replicate DMA [128, 4096] (step-0 partition fanout, rows
  ta/tb of the grid) -> modulate is ONE DVE tensor_tensor [128, 4096]/pair.
- PE: per 512-px group, 5 matmuls (4 pairs K=128 + center K=64) accumulate
  into one PSUM bank; ScalarE evicts with per-partition bias -> fp16.
"""
import numpy as np

import concourse.bacc as bacc
import concourse.bass as bass
import concourse.mybir as mybir
import concourse.tile as tile
from concourse.bass_utils import run_bass_kernel_spmd

F16 = mybir.dt.float16
F32 = mybir.dt.float32

B, C, H, W = 8, 64, 256, 256
Hp, Wp = H + 2, W + 2          # 258
NP = Hp * Wp                   # 66564
ALPHA = 8.3

CH = 4096                      # output pixels per chunk
NCHUNK = -(-NP // CH)          # 17 (out grid 69632, host slices)
WT = CH + 520                  # x2 tile width (halo for tap shifts)
SEG, SEGW = 8, CH // 8         # fd packing: [72, 512]

# line slacks (elements)
XSL, XSH = 512, 4608           # x line: reads [q0-260, q0+CH+516)
DSL, DSH = 512, 4608           # depth line
XW = XSL + NP + XSH
DW = DSL + NP + DSH
OUTW = NCHUNK * CH             # 69632

REGW = CH + 16                 # fd grid region width (per parity)
FDW = 2 * REGW

# tap id t = kh*3+kw, delta = (kh-1)*258 + (kw-1)
DELTA = [(kh - 1) * Wp + (kw - 1) for kh in range(3) for kw in range(3)]
# pairs (ta, tb): tb's shift baked into the x tile's upper half
PAIRS = [(0, 1, 0), (7, 8, 0), (2, 3, 1), (5, 6, 1)]  # (ta, tb, tile_idx)
UPPER_SHIFT = [1, 256]  # T1, T2


def _build_nc():
    nc = bacc.Bacc("TRN2", target_bir_lowering=False, debug=False, num_devices=8)
    x_line = nc.declare_dram_parameter("x_line", [C, XW], F16, isOutput=False)
    d_line = nc.declare_dram_parameter("d_line", [1, DW], F32, isOutput=False)
    wts = nc.declare_dram_parameter("wts", [128, 5 * 64], F16, isOutput=False)
    bias = nc.declare_dram_parameter("bias", [64, 1], F32, isOutput=False)
    out_l = nc.declare_dram_parameter("out_line", [C, OUTW], F16, isOutput=True)

    x_t = x_line.ap().tensor
    d_t = d_line.ap().tensor
    fd_dram = nc.dram_tensor("fd_scratch", [9, FDW], F16)
    fd_t = fd_dram.ap().tensor

    with tile.TileContext(nc) as tc:
        with (
            tc.tile_pool(name="const", bufs=1) as cpool,
            tc.tile_pool(name="xt", bufs=4) as xpool,
            tc.tile_pool(name="fdgen", bufs=3) as gpool,
            tc.tile_pool(name="frep", bufs=8) as fpool,
            tc.tile_pool(name="mmod", bufs=6) as mpool,
            tc.tile_pool(name="ost", bufs=2) as opool,
            tc.tile_pool(name="ps", bufs=8, space="PSUM") as pspool,
        ):
            wt_sb = cpool.tile([128, 5 * 64], F16, tag="w")
            nc.sync.dma_start(wt_sb[:], wts[:])
            bias_sb = cpool.tile([64, 1], F32, tag="b")
            nc.sync.dma_start(bias_sb[:], bias[:])

            for i in range(NCHUNK):
                q0 = i * CH
                xbase = XSL + q0 - 260
                # ---- x2 tiles: one 3-dim DMA each (lower x, upper x@shift)
                xts = []
                for ti in range(2):
                    xt = xpool.tile([128, WT], F16, tag="x")
                    nc.sync.dma_start(
                        xt[:],
                        bass.AP(x_t, xbase,
                                [[UPPER_SHIFT[ti], 2], [XW, 64], [1, WT]]))
                    xts.append(xt)

                # ---- fd generation (packed [72, 512]) ----
                dp = gpool.tile([72, SEGW], F32, tag="dp")
                for kh in range(3):
                    nc.sync.dma_start(
                        dp[kh * 24:(kh + 1) * 24, :],
                        bass.AP(d_t, DSL + q0 - 259 + kh * Wp,
                                [[1, 3], [SEGW, SEG], [1, SEGW]]))
                dc = gpool.tile([72, SEGW], F32, tag="dc")
                nc.sync.dma_start(
                    dc[:],
                    bass.AP(d_t, DSL + q0,
                            [[0, 9], [SEGW, SEG], [1, SEGW]]))
                df = gpool.tile([72, SEGW], F32, tag="df")
                nc.vector.tensor_tensor(df[:], dp[:], dc[:],
                                        mybir.AluOpType.subtract)
                da = gpool.tile([72, SEGW], F32, tag="da")
                nc.scalar.activation(da[:], df[:],
                                     mybir.ActivationFunctionType.Abs)
                fdp = gpool.tile([72, SEGW], F16, tag="fdp")
                nc.scalar.activation(fdp[:], da[:],
                                     mybir.ActivationFunctionType.Exp,
                                     scale=-ALPHA)
                # ---- scatter to DRAM fd grid: ONE DMA (per-parity region)
                reg = (i % 2) * REGW
                nc.sync.dma_start(
                    bass.AP(fd_t, reg,
                            [[FDW, 9], [SEGW, SEG], [1, SEGW]]),
                    fdp[:])

                # ---- per pair: replicate fd (1 DMA), modulate (1 DVE op)
                mts = []
                for (ta, tb, ti) in PAIRS:
                    fr = fpool.tile([128, CH], F16, tag="fr")
                    nc.sync.dma_start(
                        fr[:],
                        bass.AP(fd_t, ta * FDW + reg,
                                [[(tb - ta) * FDW, 2], [0, 64], [1, CH]]))
                    mt = mpool.tile([128, CH], F16, tag="m")
                    m0 = 260 + DELTA[ta]
                    nc.vector.tensor_tensor(
                        mt[:], xts[ti][:, m0:m0 + CH], fr[:],
                        mybir.AluOpType.mult)
                    mts.append(mt)

                # ---- matmuls + eviction ----
                ost = opool.tile([64, CH], F16, tag="o")
                for j in range(CH // 512):
                    ps = pspool.tile([64, 512], F32)
                    for g in range(4):
                        nc.tensor.matmul(
                            ps[:], wt_sb[:, g * 64:(g + 1) * 64],
                            mts[g][:, j * 512:(j + 1) * 512],
                            start=(g == 0), stop=False)
                    nc.tensor.matmul(
                        ps[:], wt_sb[0:64, 256:320],
                        xts[0][0:64, 260 + j * 512: 260 + (j + 1) * 512],
                        start=False, stop=True)
                    nc.scalar.activation(
                        ost[:, j * 512:(j + 1) * 512], ps[:],
                        mybir.ActivationFunctionType.Identity,
                        bias=bias_sb[:], scale=1.0)
                nc.sync.dma_start(out_l[:, q0:q0 + CH], ost[:])
    nc.compile()
    return nc


_NC_CACHE = None


def _get_nc():
    global _NC_CACHE
    if _NC_CACHE is None:
        _NC_CACHE = _build_nc()
    return _NC_CACHE


def kernel(x, depth, weight, bias):
    x = np.asarray(x, dtype=np.float32)
    depth = np.asarray(depth, dtype=np.float32)
    weight = np.asarray(weight, dtype=np.float32)
    bias_np = np.asarray(bias, dtype=np.float32)

    # host prep
    xl = np.zeros((B, C, XW), np.float16)
    xpad = np.zeros((B, C, Hp, Wp), np.float32)
    xpad[:, :, 1:257, 1:257] = x
    xl[:, :, XSL:XSL + NP] = xpad.reshape(B, C, NP).astype(np.float16)

    dl = np.zeros((B, 1, DW), np.float32)
    dpad = np.zeros((B, Hp, Wp), np.float32)
    dpad[:, 1:257, 1:257] = depth[:, 0]
    dl[:, 0, DSL:DSL + NP] = dpad.reshape(B, NP)

    wts = np.zeros((128, 5 * 64), np.float16)
    for g, (ta, tb, _) in enumerate(PAIRS):
        # lhsT[c, o] = weight[o, c, kh, kw]
        wts[0:64, g * 64:(g + 1) * 64] = \
            weight[:, :, ta // 3, ta % 3].T.astype(np.float16)
        wts[64:128, g * 64:(g + 1) * 64] = \
            weight[:, :, tb // 3, tb % 3].T.astype(np.float16)
    wts[0:64, 256:320] = weight[:, :, 1, 1].T.astype(np.float16)

    bias_col = bias_np.reshape(64, 1)

    nc = _get_nc()
    in_maps = [
        {"x_line": xl[b], "d_line": dl[b], "wts": wts, "bias": bias_col}
        for b in range(B)
    ]
    res = run_bass_kernel_spmd(nc, in_maps, list(range(B)))

    out = np.empty((B, C, H, W), np.float32)
    for b in range(B):
        ol = res.results[b]["out_line"][:, :NP].astype(np.float32)
        out[b] = ol.reshape(C, Hp, Wp)[:, 1:257, 1:257]
    return out
